# revision 5
# baseline (speedup 1.0000x reference)
"""Multi-head attention block on 8 Trainium2 NeuronCores — v9.

v9 vs v8: V projection restructured from 512 tiny [x-chunk stationary,
128-row] matmuls to the V^T form — 128 [wv stationary, 512-row] matmuls plus
64 PE transposes — cutting ~320 stationary loads (~26.5ns each, measured
unpipelined) and ~320 instruction issues per iteration.


v8 vs v5: weight DMAs (wq8/wk8/wv/wo/bq) hoisted out of the hwloop body —
weights are iteration-invariant, so steady-state reps only re-DMA x. The
freed sync/scalar/gpsimd queue slots at body start go to the x slabs.


v5 vs v4: Q/K projections use fp8-e4m3 DoubleRow matmuls (2 contraction rows
per cycle; contraction = 4 chunks of 256). The softmax scale is NOT folded
into wq (fp8 would go subnormal); instead host scales wq/wk by 64 into prime
e4m3 range and the exp activation applies the combined 0.125/64^2 scale for
free. fp8 x/wq/wk use the same [p, ko, .] layout as bf16, read as adjacent
ko-chunk pairs — no special packing. bf16 x stays resident for the V
projection (stationary side).


v4 vs v3:
  - DMA order: first weight chunk + first x slab land first so the first
    projection matmul starts ~3us in instead of ~18us.
  - out-projection chunks for t-slice ts are interleaved into the (otherwise
    ACT-bound) attention of phase 3, q-tile ts+1, as soon as their ot columns
    are normalized; the rest drains after.
  - y DMAs issue on sync/gpsimd only (scalar's sequencer is busy with exps).

v3 vs v2: projections for phase m+1 are interleaved into the attention inner
loop of phase m (the attention stretch is ACT-bound — one [128,1024] exp per
k-tile takes ~1.04us vs ~0.85us of PE matmuls — so the PE slack absorbs the
projection matmuls). attn@V lags scores by 2 k-tiles so exp latency is fully
hidden. PSUM: 2 score tiles (4 banks) + 2 proj tiles (2 banks) + poA/poB
(2 banks) = 8 banks.

See kernel_v2.py docstring for the overall decomposition and layouts.
"""

import sys

for _p in ("/opt/trn_rl_repo",):
    if _p not in sys.path:
        sys.path.insert(0, _p)

import numpy as np

import concourse.bass as bass
import concourse.tile as tile
from concourse import bacc, mybir
from concourse.bass_utils import run_bass_kernel_spmd
from concourse.masks import make_identity

F32 = mybir.dt.float32
BF16 = mybir.dt.bfloat16
FP8 = mybir.dt.float8e4
DR = mybir.MatmulPerfMode.DoubleRow
EXP = mybir.ActivationFunctionType.Exp

W8SCALE = 64.0                      # host premultiplies wq/wk by this
SCALE8 = 0.125 / (W8SCALE * W8SCALE)  # head_dim**-0.5 / W8SCALE^2, in exp


DIM = 1024
T = 2048
P = 128
LH = 512           # head dims per core (8 heads x 64)
NPH = 4            # head-pair phases per core
KD = DIM // P      # 8 contraction tiles over the model dim
NT = T // P        # 16 key tiles
NQ = T // 512      # 4 query column tiles
HD = 64

_CACHE: dict = {}


def build_nc(reps=1, upto="full", hwloop=False):
    nc = bacc.Bacc(None, target_bir_lowering=False)

    xT = nc.dram_tensor("xT", [DIM, T], BF16, kind="ExternalInput")
    xT8 = nc.dram_tensor("xT8", [DIM, T], FP8, kind="ExternalInput")
    wqT8 = nc.dram_tensor("wqT8", [DIM, LH], FP8, kind="ExternalInput")
    wkT8 = nc.dram_tensor("wkT8", [DIM, LH], FP8, kind="ExternalInput")
    wvT = nc.dram_tensor("wvT", [DIM, LH], BF16, kind="ExternalInput")
    woT = nc.dram_tensor("woT", [LH, DIM], BF16, kind="ExternalInput")
    bq = nc.dram_tensor("bq", [LH], F32, kind="ExternalInput")
    yT = nc.dram_tensor("yT", [DIM, T], BF16, kind="ExternalOutput")

    xT_t = xT.rearrange("(ko p) t -> p ko t", p=P)      # [128, 8, 2048]
    xT8_t = xT8.rearrange("(ko p) t -> p ko t", p=P)
    wqT8_t = wqT8.rearrange("(ko p) m -> p ko m", p=P)  # [128, 8, 512]
    wkT8_t = wkT8.rearrange("(ko p) m -> p ko m", p=P)
    wvT_t = wvT.rearrange("(ko p) m -> p ko m", p=P)
    woT_t = woT.rearrange("(mo p) n -> p mo n", p=P)    # [128, 4, 1024]
    bq_t = bq.rearrange("(mo p) -> p mo", p=P)          # [128, 4]
    yT_t = yT.rearrange("(no p) t -> p no t", p=P)      # [128, 8, 2048]

    with (
        tile.TileContext(nc) as tc,
        tc.tile_pool(name="const", bufs=1) as const_pool,
        tc.tile_pool(name="p", bufs=4) as p_pool,
        tc.tile_pool(name="ep", bufs=2) as ep_pool,
        tc.tile_pool(name="ps", bufs=1, space="PSUM") as psum,
    ):
        # ---- iteration-invariant setup: weights resident across reps ----
        x_sb = const_pool.tile([P, KD, T], BF16)
        x8_sb = const_pool.tile([P, KD, T], FP8)
        wq_sb = const_pool.tile([P, KD, LH], FP8)
        wk_sb = const_pool.tile([P, KD, LH], FP8)
        wv_sb = const_pool.tile([P, KD, LH], BF16)
        wo_sb = const_pool.tile([P, NPH, DIM], BF16)
        bq_sb = const_pool.tile([P, NPH], F32)
        nc.scalar.dma_start(wq_sb[:], wqT8_t[:])
        nc.gpsimd.dma_start(wk_sb[:], wkT8_t[:])
        nc.scalar.dma_start(wv_sb[:], wvT_t[:])
        nc.gpsimd.dma_start(wo_sb[:], woT_t[:])
        nc.scalar.dma_start(bq_sb[:], bq_t[:])

        qt = const_pool.tile([P, NPH, T], BF16)
        kt = const_pool.tile([P, NPH, T], BF16)
        # V with a ones column per head: attn@V row 64 = softmax denom; the
        # ones columns are never overwritten (copies skip cols 64/129)
        v_sb = const_pool.tile([P, NPH, NT, 130], BF16)
        nc.gpsimd.memset(v_sb[:, :, :, 64:65], 1.0)
        nc.gpsimd.memset(v_sb[:, :, :, 129:130], 1.0)
        # normalized attention output, [dh within pair, phase, t]
        ot_sb = const_pool.tile([P, NPH, T], BF16)
        ident_f = const_pool.tile([P, P], F32)
        make_identity(nc, ident_f[:])
        ident = const_pool.tile([P, P], BF16)
        nc.vector.tensor_copy(ident[:], ident_f[:])

        def _body():
            # ---- per-iteration activation input ----
            # x8 slabs first (Q/K projections), bf16 x after (V projection
            # runs after Q/K within each phase's generator).
            for sl in range(4):
                tsl = slice(sl * 512, (sl + 1) * 512)
                engs2 = [nc.sync, nc.scalar]
                engs2[sl % 2].dma_start(x8_sb[:, :, tsl], xT8_t[:, :, tsl])
            for sl in range(4):
                tsl = slice(sl * 512, (sl + 1) * 512)
                engs2 = [nc.gpsimd, nc.sync]
                engs2[sl % 2].dma_start(x_sb[:, :, tsl], xT_t[:, :, tsl])

            def gen_proj(m):
                """Generator: each next() emits exactly one projection matmul
                for phase m; DVE copies + tile rotation happen at chunk
                boundaries. Drained fully for m=0, interleaved into the
                attention of phase m-1 otherwise."""
                msl = slice(m * P, (m + 1) * P)
                for n in range(NQ):
                    nsl = slice(n * 512, (n + 1) * 512)
                    ps = psum.tile([P, 512], F32, tag="pj", bufs=2, name="psq")
                    for c in range(KD // 2):
                        ksl = slice(2 * c, 2 * c + 2)
                        nc.tensor.matmul(
                            ps[:], wq_sb[:, ksl, msl], x8_sb[:, ksl, nsl],
                            start=(c == 0), stop=(c == KD // 2 - 1),
                            perf_mode=DR,
                        )
                        yield
                    nc.vector.tensor_scalar_add(
                        qt[:, m, nsl], ps[:], bq_sb[:, m : m + 1]
                    )
                    ps = psum.tile([P, 512], F32, tag="pj", bufs=2, name="psk")
                    for c in range(KD // 2):
                        ksl = slice(2 * c, 2 * c + 2)
                        nc.tensor.matmul(
                            ps[:], wk_sb[:, ksl, msl], x8_sb[:, ksl, nsl],
                            start=(c == 0), stop=(c == KD // 2 - 1),
                            perf_mode=DR,
                        )
                        yield
                    nc.vector.tensor_copy(kt[:, m, nsl], ps[:])
                # V^T (wv stationary, 512-row moving), then PE-transpose each
                # [128,128] tile into v_sb's [t, dh] layout — 4x fewer
                # stationary loads than x-stationary direct projection
                for n in range(NQ):
                    nsl = slice(n * 512, (n + 1) * 512)
                    sv = psum.tile([P, 512], F32, tag="pj", bufs=2, name="psv")
                    for k in range(KD):
                        nc.tensor.matmul(
                            sv[:], wv_sb[:, k, msl], x_sb[:, k, nsl],
                            start=(k == 0), stop=(k == KD - 1),
                        )
                        yield
                    vt = ep_pool.tile([P, 512], BF16, tag="vt", bufs=2)
                    nc.vector.tensor_copy(vt[:], sv[:])
                    pt = psum.tile([P, 512], F32, tag="pj", bufs=2, name="ptr")
                    ptv = pt.bitcast(BF16)
                    for i in range(4):
                        t = n * 4 + i
                        csl = slice(i * P, (i + 1) * P)
                        nc.tensor.transpose(
                            ptv[:, csl], vt[:, csl], ident[:]
                        )
                        yield
                        nc.vector.tensor_copy(
                            v_sb[:, m, t, 0:64], ptv[:, i * P : i * P + 64]
                        )
                        nc.vector.tensor_copy(
                            v_sb[:, m, t, 65:129], ptv[:, i * P + 64 : (i + 1) * P]
                        )

            _DONE = object()

            def drain(g, k=1 << 30):
                for _ in range(k):
                    if next(g, _DONE) is _DONE:
                        break

            ydma = [nc.sync, nc.gpsimd]

            def gen_outproj(ts):
                """Out-projection for t-slice ts; interleavable once
                ot[:, :, tsl] is fully normalized (after attention(3, ts))."""
                tsl = slice(ts * 512, (ts + 1) * 512)
                for nt in range(KD):
                    ntsl = slice(nt * P, (nt + 1) * P)
                    st = psum.tile([P, 512], F32, tag="pj", bufs=2, name="psy")
                    for m2 in range(NPH):
                        nc.tensor.matmul(
                            st[:], wo_sb[:, m2, ntsl], ot_sb[:, m2, tsl],
                            start=(m2 == 0), stop=(m2 == NPH - 1),
                        )
                        yield
                    ysb = ep_pool.tile([P, 512], BF16, tag="y", bufs=3)
                    nc.vector.tensor_copy(ysb[:], st[:])
                    ydma[(nt + ts) % 2].dma_start(yT_t[:, nt, tsl], ysb[:])

            gen_out = [gen_outproj(ts) for ts in range(NQ)]

            # phase-0 projections upfront
            g0 = gen_proj(0)
            drain(g0)

            # ---- attention, with phase m+1 projections (or, in phase 3,
            # ---- ready out-projection chunks) interleaved ----
            for m in range(NPH if upto != "proj" else 0):
                gp = gen_proj(m + 1) if m + 1 < NPH else iter(())
                for q in range(NQ):
                    if m == NPH - 1 and upto == "full" and q >= 1:
                        g = gen_out[q - 1]
                    else:
                        g = gp
                    qsl = slice(q * 512, (q + 1) * 512)
                    poA = psum.tile([65, 512], F32, tag="poA", bufs=1)
                    poB = psum.tile([65, 512], F32, tag="poB", bufs=1)

                    def S(k):
                        ksl = slice(k * P, (k + 1) * P)
                        st = psum.tile([P, 1024], F32, tag="s", bufs=2)
                        nc.tensor.matmul(
                            st[:, 0:512], kt[0:HD, m, ksl], qt[0:HD, m, qsl],
                            start=True, stop=True,
                        )
                        nc.tensor.matmul(
                            st[:, 512:1024], kt[HD:P, m, ksl], qt[HD:P, m, qsl],
                            start=True, stop=True,
                        )
                        p = p_pool.tile([P, 1024], BF16, tag="p")
                        nc.scalar.activation(p[:], st[:], EXP, scale=SCALE8)
                        return p

                    def A(k, p):
                        nc.tensor.matmul(
                            poA[:], v_sb[:, m, k, 0:65], p[:, 0:512],
                            start=(k == 0), stop=(k == NT - 1),
                        )
                        nc.tensor.matmul(
                            poB[:], v_sb[:, m, k, 65:130], p[:, 512:1024],
                            start=(k == 0), stop=(k == NT - 1),
                        )

                    # software pipeline: scores lead attn@V by 2 k-tiles
                    pend = [S(0)]
                    drain(g, 3)
                    pend.append(S(1))
                    drain(g, 3)
                    for k in range(NT):
                        if k + 2 < NT:
                            pend.append(S(k + 2))
                        drain(g, 3)
                        A(k, pend.pop(0))
                    if upto == "scores":
                        continue
                    # normalize: row 64 is the softmax denominator
                    for h, po in ((0, poA), (1, poB)):
                        rc = ep_pool.tile([1, 512], F32, tag="rc")
                        nc.vector.reciprocal(rc[0:1, :], po[64:65, :])
                        bc = ep_pool.tile([HD, 512], F32, tag="bc")
                        nc.gpsimd.partition_broadcast(bc[:], rc[0:1, :])
                        nc.vector.tensor_mul(
                            ot_sb[h * HD : (h + 1) * HD, m, qsl], po[0:HD, :], bc[:]
                        )
                drain(gp)

            # ---- remaining out-projection chunks ----
            if upto == "full":
                for ts in range(NQ):
                    drain(gen_out[ts])

        if hwloop:
            with tc.For_i(0, reps, 1):
                _body()
        else:
            for _rep in range(reps):
                _body()

    nc.finalize()
    return nc


def _get_nc():
    if "nc" not in _CACHE:
        _CACHE["nc"] = build_nc()
    return _CACHE["nc"]


def make_in_maps(x, wq, bq, wk, bk, wv, bv, wo, bo):
    x = np.asarray(x, np.float32)
    wq, bq = np.asarray(wq, np.float32), np.asarray(bq, np.float32)
    wk = np.asarray(wk, np.float32)
    wv = np.asarray(wv, np.float32)
    wo = np.asarray(wo, np.float32)
    scale = np.float32(HD ** -0.5)

    del scale  # applied on-device via the exp activation's scale (SCALE8)
    wqT8 = np.ascontiguousarray(wq.T) * np.float32(W8SCALE)
    bq8 = bq * np.float32(W8SCALE)
    wkT8 = np.ascontiguousarray(wk.T) * np.float32(W8SCALE)
    wvT = np.ascontiguousarray(wv.T)
    woT = np.ascontiguousarray(wo.T)

    npdt = mybir.dt.np(BF16)
    np8 = mybir.dt.np(FP8)
    in_maps = []
    for c in range(8):
        b, hg = c // 2, c % 2
        cols = slice(hg * LH, (hg + 1) * LH)
        xTb = np.ascontiguousarray(x[b].T)
        in_maps.append(
            {
                "xT": xTb.astype(npdt),
                "xT8": xTb.astype(np8),
                "wqT8": np.ascontiguousarray(wqT8[:, cols]).astype(np8),
                "wkT8": np.ascontiguousarray(wkT8[:, cols]).astype(np8),
                "wvT": np.ascontiguousarray(wvT[:, cols]).astype(npdt),
                "woT": np.ascontiguousarray(woT[cols, :]).astype(npdt),
                "bq": np.ascontiguousarray(bq8[cols]),
            }
        )
    return in_maps


def kernel(x, wq, bq, wk, bk, wv, bv, wo, bo, _results_hook=None):
    in_maps = make_in_maps(x, wq, bq, wk, bk, wv, bv, wo, bo)
    nc = _get_nc()
    res = run_bass_kernel_spmd(nc, in_maps, list(range(8)))
    if _results_hook is not None:
        _results_hook(res)

    wo_np = np.asarray(wo, np.float32)
    const = np.asarray(bo, np.float32) + np.asarray(bv, np.float32) @ wo_np.T
    y = np.empty((4, T, DIM), np.float32)
    for b in range(4):
        y[b] = res.results[2 * b]["yT"].astype(np.float32).T
        y[b] += res.results[2 * b + 1]["yT"].astype(np.float32).T
        y[b] += const
    return y
